# revision 6
# baseline (speedup 1.0000x reference)
"""Paged KV-cache append kernel for Trainium2 (8 NeuronCores).

Problem: scatter new k/v tokens [T=32768, H=8, D=128] into a paged pool
kv_cache [NPAGES=4096, 2, PAGE=16, H, D] per flashinfer append semantics.

Strategy (page-sharded, gather formulation):
  - Each core owns NPAGES/8 = 512 destination pages -> its output slice is
    contiguous: kv_cache[c*512:(c+1)*512]  (64 MiB).
  - One "row" = one (page, k-or-v) half-page = PAGE*H*D = 16384 f32 = 64 KiB,
    contiguous both in the cache layout and in the token stream (16
    consecutive tokens fill one page's slots 0..15 for the given inputs).
  - Host computes, for every output row, the source row id inside a
    per-core source pool  src = [k-groups (2048) | v-groups (2048) |
    old-cache rows for owned pages (1024)]  -> a 1024-entry int32 index.
  - Device: 8x { indirect DMA gather (DRAM->SBUF, 128 rows x 64KiB) ;
    direct DMA store (SBUF->DRAM, contiguous 8 MiB) }, double-buffered.
    Every core moves exactly 64 MiB in + 64 MiB out -> perfectly balanced,
    ~128 MiB of HBM traffic per core (the memory roofline for this op).
"""

import numpy as np

import concourse.bass as bass
import concourse.mybir as mybir
from concourse.bass_utils import run_bass_kernel_spmd
from concourse.tile import TileContext

# ---- problem shapes (hardcoded per contract) ----
T, H, D = 32768, 8, 128
PAGE = 16
NPAGES = 4096
NCORES = 8
PPC = NPAGES // NCORES          # 512 pages per core
ROW = PAGE * H * D              # 16384 f32 per (page, kv) row = 64 KiB
NGRP = T // PAGE                # 2048 token groups (one per written page)
SRC_ROWS = 2 * NGRP + 2 * PPC   # 5120
OUT_ROWS = 2 * PPC              # 1024 rows per core
P = 128                         # SBUF partitions
NTILES = OUT_ROWS // P          # 8 gather/store tiles per core

# set by test harness to collect a profile; grading path leaves these alone
TRACE = False
LAST = None

_program = None


def _build_program():
    # Raw bass (no Tile): the walrus backend only allows ONE sync-wait per
    # DMA/CTRL instruction, which Tile's auto-sync exceeds. A single gpsimd
    # sequencer with standalone wait_ge instructions and one counting
    # semaphore sidesteps that entirely. Every DMA increments sem by 16;
    # waiting for "all DMAs issued so far" is a conservative superset of the
    # true deps and costs nothing extra (the kernel is HBM-bound either way).
    nc = bass.Bass()
    src = nc.dram_tensor("src", [SRC_ROWS, ROW], mybir.dt.float32,
                         kind="ExternalInput")
    # already transposed host-side: [partition, iteration]
    idx = nc.dram_tensor("idx", [P, NTILES], mybir.dt.int32,
                         kind="ExternalInput")
    out = nc.dram_tensor("out", [OUT_ROWS, ROW], mybir.dt.float32,
                         kind="ExternalOutput")
    with nc.Block() as block, \
         nc.semaphore("sem") as sem, \
         nc.sbuf_tensor("itile", [P, NTILES], mybir.dt.int32) as itile, \
         nc.sbuf_tensor("b0", [P, ROW], mybir.dt.float32) as b0, \
         nc.sbuf_tensor("b1", [P, ROW], mybir.dt.float32) as b1:

        @block.gpsimd
        def _(g):
            bufs = [b0, b1]
            g.dma_start(out=itile[:, :], in_=idx[:, :]).then_inc(sem, 16)
            g.wait_ge(sem, 16)
            g.indirect_dma_start(
                out=bufs[0][:, :], out_offset=None, in_=src[:, :],
                in_offset=bass.IndirectOffsetOnAxis(ap=itile[:, 0:1], axis=0),
            ).then_inc(sem, 16)
            g.wait_ge(sem, 32)
            done = 32
            for i in range(NTILES):
                g.dma_start(out=out[i * P:(i + 1) * P, :],
                            in_=bufs[i % 2][:, :]).then_inc(sem, 16)
                if i < NTILES - 1:
                    g.indirect_dma_start(
                        out=bufs[(i + 1) % 2][:, :], out_offset=None,
                        in_=src[:, :],
                        in_offset=bass.IndirectOffsetOnAxis(
                            ap=itile[:, i + 1:i + 2], axis=0),
                    ).then_inc(sem, 16)
                    done += 32
                    g.wait_ge(sem, done)
            g.wait_ge(sem, 16 * (1 + 2 * NTILES))
    return nc


def kernel(k, v, kv_cache, kv_append_indptr, kv_page_indices,
           kv_page_indptr, kv_page_lastlen, page_size):
    global _program, LAST
    k = np.ascontiguousarray(np.asarray(k), dtype=np.float32)
    v = np.ascontiguousarray(np.asarray(v), dtype=np.float32)
    kv_cache = np.asarray(kv_cache)
    ai = np.asarray(kv_append_indptr).astype(np.int64)
    pidx = np.asarray(kv_page_indices).astype(np.int64)
    pi = np.asarray(kv_page_indptr).astype(np.int64)
    lastlen = np.asarray(kv_page_lastlen).astype(np.int64)
    page_size = int(page_size)
    assert page_size == PAGE and k.shape == (T, H, D)

    # per-token destination (general reference semantics, vectorized)
    t = np.arange(T, dtype=np.int64)
    b = np.searchsorted(ai, t, side="right") - 1
    num_new = ai[b + 1] - ai[b]
    num_pages = pi[b + 1] - pi[b]
    seq_len = (num_pages - 1) * page_size + lastlen[b]
    pos = seq_len - num_new + (t - ai[b])
    page = pidx[pi[b] + pos // page_size]
    slot = pos % page_size

    # this kernel relies on 16-token groups mapping to whole pages
    pg = page.reshape(NGRP, PAGE)
    sg = slot.reshape(NGRP, PAGE)
    assert (sg == np.arange(PAGE)).all() and (pg == pg[:, :1]).all(), \
        "unaligned append not supported"
    grp_page = pg[:, 0]                      # dst page of token group g

    g_of_page = np.full(NPAGES, -1, np.int64)
    g_of_page[grp_page] = np.arange(NGRP)    # inverse permutation

    k2 = k.reshape(NGRP, ROW)
    v2 = v.reshape(NGRP, ROW)
    cache_base = 2 * NGRP
    loc2 = 2 * np.arange(PPC, dtype=np.int64)
    in_maps = []
    for c in range(NCORES):
        p0 = c * PPC
        g = g_of_page[p0:p0 + PPC]           # [512]
        written = g >= 0
        idx = np.empty(OUT_ROWS, np.int32)
        idx[0::2] = np.where(written, g, cache_base + loc2)
        idx[1::2] = np.where(written, NGRP + g, cache_base + loc2 + 1)
        cache_c = np.ascontiguousarray(kv_cache[p0:p0 + PPC],
                                       dtype=np.float32).reshape(2 * PPC, ROW)
        src_c = np.concatenate([k2, v2, cache_c], axis=0)
        in_maps.append({"src": src_c,
                        "idx": np.ascontiguousarray(idx.reshape(NTILES, P).T)})

    if _program is None:
        _program = _build_program()
    res = run_bass_kernel_spmd(_program, in_maps, list(range(NCORES)),
                               trace=TRACE)
    LAST = res
    outs = [res.results[c]["out"].reshape(PPC, 2, PAGE, H, D)
            for c in range(NCORES)]
    return np.concatenate(outs, axis=0)
